# revision 15
# baseline (speedup 1.0000x reference)
"""BiAttention Trainium2 kernel (8 NeuronCores, batch-parallel).

Problem (per batch element b, 8 of them -> one per core):
    A_proj = A @ W_A + b_A            [2048, 64]
    B_proj = B @ W_B + b_B            [2048, 64]
    S      = A_proj @ B_proj^T        [2048, 2048]
    A_star = softmax(S, axis=-1) @ B  [2048, 768]
    B_star = softmax(S, axis=0)^T @ A [2048, 768]

Key algebra used on-device (S is small: |S| < ~30, so exp(S) is safe in
f32/bf16 without max-subtraction):
    E = exp(S)
    A_star = diag(1/rowsum(E)) . (E @ B)
    B_star = diag(1/colsum(E)) . (E^T @ A)
rowsum/colsum are obtained for free by augmenting the moving operands
with a ones-column (E @ [B | 1] gives the row sums in the last column).

E is never materialized in full: score panels are recomputed per
512-wide output stripe (K=64 contraction - cheap) directly from the
projections, exp'd into bf16 packs, and immediately consumed as the
stationary operand of the big matmuls. Pack production for stripe u+1
is emitted ahead of stripe u's accumulation so ScalarE exp latency
hides under TensorE work.
"""

import sys

if "/opt/trn_rl_repo" not in sys.path:
    sys.path.insert(0, "/opt/trn_rl_repo")

import numpy as np

import concourse.bass as bass
import concourse.mybir as mybir
import concourse.tile as tile
from concourse import bacc
from concourse.bass import ts
from concourse.bass_utils import run_bass_kernel_spmd
from concourse.masks import make_identity

F32 = mybir.dt.float32
BF16 = mybir.dt.bfloat16
AF = mybir.ActivationFunctionType

L = 2048          # sequence length (both La and Lb)
D = 768           # model dim
H = 64            # projection dim
NT = L // 128     # 16 row/col tiles of 128
KD = D // 128     # 6 contraction tiles for the projections
NSUP = L // 512   # 4 supers (512-wide output stripes)
DP = D + 1        # moving operand width with the ones column

N_CORES = 8

_CACHE = {}


def _build():
    nc = bacc.Bacc("TRN2", target_bir_lowering=False, debug=False,
                   num_devices=N_CORES)
    A_d = nc.dram_tensor("A", [L, D], F32, kind="ExternalInput").ap()
    B_d = nc.dram_tensor("B", [L, D], F32, kind="ExternalInput").ap()
    WA_d = nc.dram_tensor("W_A", [D, H], F32, kind="ExternalInput").ap()
    WB_d = nc.dram_tensor("W_B", [D, H], F32, kind="ExternalInput").ap()
    bA_d = nc.dram_tensor("b_A", [H, 1], F32, kind="ExternalInput").ap()
    bB_d = nc.dram_tensor("b_B", [H, 1], F32, kind="ExternalInput").ap()
    AS_d = nc.dram_tensor("A_star", [L, D], F32, kind="ExternalOutput").ap()
    BS_d = nc.dram_tensor("B_star", [L, D], F32, kind="ExternalOutput").ap()

    with tile.TileContext(nc) as tc:
        with (
            tc.tile_pool(name="stage", bufs=2) as pstage,
            tc.tile_pool(name="mov", bufs=1) as pmov,
            tc.tile_pool(name="proj", bufs=1) as pproj,
            tc.tile_pool(name="pack", bufs=2) as ppack,
            tc.tile_pool(name="outp", bufs=4) as pout,
            tc.tile_pool(name="psum", bufs=2, space="PSUM") as pps,
        ):
            # identity for TensorE-based transposition
            ident = pmov.tile([128, 128], BF16, tag="ident", name="ident")
            make_identity(nc, ident)

            def load_weights():
                for side, (W_dram, b_dram) in (
                    ("B", (WB_d, bB_d)), ("A", (WA_d, bA_d))
                ):
                    wb = pmov.tile([128, KD, H], BF16, tag=f"w{side}",
                                   name=f"w{side}b")
                    nc.gpsimd.dma_start(
                        out=wb, in_=W_dram.rearrange("(k p) h -> p k h", p=128)
                    )
                    bt = pmov.tile([H, 1], F32, tag=f"b{side}",
                                   name=f"b{side}sb")
                    nc.gpsimd.dma_start(out=bt, in_=b_dram)
                    w_sb[side] = wb
                    b_sb[side] = bt

            w_sb = {}
            b_sb = {}

            # ---- load(+cast) and transpose via TensorE ----
            aug = {}
            projT = {}
            mts = {}
            dram = {"A": A_d, "B": B_d}
            for side in ("A", "B"):
                mts[side] = pmov.tile([128, NT * KD, 128], BF16,
                                      tag=f"t{side}", name=f"{side}_T")
                aug[side] = pmov.tile([128, NT, DP], BF16, tag=f"aug{side}",
                                      name=f"{side}_aug")

            def prep_unit(side, u):
                # 2-tile load unit (i = 2u, 2u+1): one casting DMA
                # f32 DRAM -> bf16 SBUF (SWDGE only); per-unit staging keeps
                # loads parallel (no false deps on big tensors)
                stg = pstage.tile([128, 2, D], BF16, tag="stg", bufs=6,
                                  name=f"stg{side}{u}")
                nc.gpsimd.dma_start(
                    out=stg,
                    in_=dram[side][u * 256:(u + 1) * 256, :].rearrange(
                        "(t p) d -> p t d", p=128
                    ),
                )
                for t in range(2):
                    i = 2 * u + t
                    # transpose the 6 blocks on TensorE: psum <- block.T
                    ps = pps.tile([128, 1024], F32, tag="accum",
                                  name=f"pstr{side}{i}")
                    for j in range(KD):
                        nc.tensor.matmul(ps[:, ts(j, 128)],
                                         stg[:, t, ts(j, 128)],
                                         ident, start=True, stop=True)
                    nc.vector.tensor_copy(
                        out=mts[side][:, i * KD:(i + 1) * KD, :],
                        in_=ps[:, 0:KD * 128],
                    )
                    nc.scalar.copy(out=aug[side][:, i, 0:D], in_=stg[:, t, :])

            def proj_chunk(side, n):
                # projT[h, s] = sum_d W[d,h] M^T[d,s] over 1024-wide chunk
                mtv = mts[side].rearrange("p (i j) q -> p i j q", j=KD)
                ps = pps.tile([128, 1024], F32, tag="spack",
                              name=f"psproj{side}{n}")
                for nn in range(2):     # 512-wide matmuls
                    i0 = n * 8 + nn * 4
                    for k in range(KD):
                        nc.tensor.matmul(
                            ps[:H, ts(nn, 512)],
                            w_sb[side][:, k, :],
                            mtv[:, i0:i0 + 4, k, :],
                            start=(k == 0), stop=(k == KD - 1),
                        )
                nc.scalar.activation(
                    out=projT[side][0:H, ts(n, 1024)], in_=ps[:H, :],
                    func=AF.Identity, bias=b_sb[side], scale=1.0,
                )
                # duplicate into partitions 64:128 for row-packed S matmuls
                nc.sync.dma_start(out=projT[side][64:128, ts(n, 1024)],
                                  in_=projT[side][0:H, ts(n, 1024)])

            for side in ("A", "B"):
                # rows 0:64 written by proj activation; rows 64:128 get a
                # duplicate (via SBUF->SBUF DMA) so K=64 score matmuls can be
                # row-packed two-at-a-time with tile_position (0,0)/(64,0)
                projT[side] = pproj.tile([128, L], BF16, tag=f"p{side}",
                                         name=f"{side}_projT")

            for u in range(4):
                prep_unit("A", u)
            load_weights()
            for u in range(8):
                prep_unit("B", u)
            nc.vector.memset(aug["B"][:, :, D:DP], 1.0)
            proj_chunk("B", 0)
            proj_chunk("B", 1)
            proj_chunk("A", 0)
            for u in range(4, 8):
                prep_unit("A", u)
            nc.vector.memset(aug["A"][:, :, D:DP], 1.0)
            proj_chunk("A", 1)

            # ---- main: per 512-wide output stripe, software-pipelined ----
            # dirn "A": produce A_star rows; panels are E'[t, s-stripe]
            #   (lhsT = B_projT tiles, rhs = A_projT stripe), moving = B_aug
            # dirn "B": produce B_star rows; panels are E[s, t-stripe]
            #   (lhsT = A_projT tiles, rhs = B_projT stripe), moving = A_aug
            work = [("A", u) for u in range(NSUP)] + \
                   [("B", u) for u in range(NSUP)]
            spec = {
                "A": (projT["B"], projT["A"], aug["B"], AS_d),
                "B": (projT["A"], projT["B"], aug["A"], BS_d),
            }
            packs = {}

            def emit_pack(w):
                dirn, u = w
                pT_l, pT_r, _, _ = spec[dirn]
                pk = ppack.tile([128, NT * 512], BF16, tag="pack",
                                name=f"pk{dirn}{u}")
                for jp in range(NT // 2):
                    ps = pps.tile([128, 1024], F32, tag="spack",
                                  name=f"pss{dirn}{u}{jp}")
                    for h2 in range(2):
                        # row-packed pair: K=64 matmuls in rows 0:64 / 64:128
                        j = jp * 2 + h2
                        base = h2 * 64
                        nc.tensor.matmul(
                            ps[:, ts(h2, 512)],
                            pT_l[base:base + H, ts(j, 128)],
                            pT_r[base:base + H, ts(u, 512)],
                            start=True, stop=True,
                            tile_position=(base, 0),
                        )
                    nc.scalar.activation(
                        out=pk[:, jp * 1024:(jp + 1) * 1024], in_=ps,
                        func=AF.Exp,
                    )
                packs[w] = pk

            def emit_accum(w):
                dirn, u = w
                _, _, mv, out_d = spec[dirn]
                pk = packs.pop(w)
                for ii in range(4):
                    pa = pps.tile([128, 1024], F32, tag="accum",
                                  name=f"pa{dirn}{u}{ii}")
                    for j in range(NT):
                        lhs = pk[:, j * 512 + ii * 128:
                                 j * 512 + ii * 128 + 128]
                        nc.tensor.matmul(
                            pa[:, 0:512], lhs, mv[:, j, 0:512],
                            start=(j == 0), stop=(j == NT - 1),
                        )
                        nc.tensor.matmul(
                            pa[:, 512:DP], lhs, mv[:, j, 512:DP],
                            start=(j == 0), stop=(j == NT - 1),
                        )
                    rinv = pout.tile([128, 1], F32, tag="rinv",
                                     name=f"ri{dirn}{u}{ii}")
                    nc.vector.reciprocal(out=rinv, in_=pa[:, D:DP])
                    ot = pout.tile([128, D], F32, tag="ot",
                                   name=f"ot{dirn}{u}{ii}")
                    nc.vector.tensor_scalar_mul(ot, pa[:, 0:D], rinv)
                    nc.sync.dma_start(
                        out=out_d[ts(u * 4 + ii, 128), :], in_=ot
                    )

            emit_pack(work[0])
            for idx, w in enumerate(work):
                if idx + 1 < len(work):
                    emit_pack(work[idx + 1])
                emit_accum(w)

    nc.compile()
    return nc


def _get_nc():
    if "nc" not in _CACHE:
        _CACHE["nc"] = _build()
    return _CACHE["nc"]


def _run(inputs, trace=False):
    nc = _get_nc()
    A = np.ascontiguousarray(np.asarray(inputs["A"], dtype=np.float32))
    B = np.ascontiguousarray(np.asarray(inputs["B"], dtype=np.float32))
    W_A = np.ascontiguousarray(np.asarray(inputs["W_A"], dtype=np.float32))
    W_B = np.ascontiguousarray(np.asarray(inputs["W_B"], dtype=np.float32))
    b_A = np.asarray(inputs["b_A"], dtype=np.float32).reshape(H, 1)
    b_B = np.asarray(inputs["b_B"], dtype=np.float32).reshape(H, 1)
    in_maps = [
        {
            "A": A[c], "B": B[c],
            "W_A": W_A, "W_B": W_B,
            "b_A": b_A, "b_B": b_B,
        }
        for c in range(N_CORES)
    ]
    res = run_bass_kernel_spmd(nc, in_maps, list(range(N_CORES)), trace=trace)
    A_star = np.stack([res.results[c]["A_star"] for c in range(N_CORES)])
    B_star = np.stack([res.results[c]["B_star"] for c in range(N_CORES)])
    return A_star, B_star, res


def kernel(**inputs):
    A_star, B_star, _ = _run(inputs)
    return A_star, B_star


# revision 16
# speedup vs baseline: 1.1672x; 1.1672x over previous
"""BiAttention Trainium2 kernel (8 NeuronCores, batch-parallel).

Problem (per batch element b, 8 of them -> one per core):
    A_proj = A @ W_A + b_A            [2048, 64]
    B_proj = B @ W_B + b_B            [2048, 64]
    S      = A_proj @ B_proj^T        [2048, 2048]
    A_star = softmax(S, axis=-1) @ B  [2048, 768]
    B_star = softmax(S, axis=0)^T @ A [2048, 768]

Key algebra used on-device (S is small: |S| < ~30, so exp(S) is safe in
f32/bf16 without max-subtraction):
    E = exp(S)
    A_star = diag(1/rowsum(E)) . (E @ B)
    B_star = diag(1/colsum(E)) . (E^T @ A)
rowsum/colsum are obtained for free by augmenting the moving operands
with a ones-column (E @ [B | 1] gives the row sums in the last column).

E is never materialized in full: score panels are recomputed per
512-wide output stripe (K=64 contraction - cheap) directly from the
projections, exp'd into bf16 packs, and immediately consumed as the
stationary operand of the big matmuls. Pack production for stripe u+1
is emitted ahead of stripe u's accumulation so ScalarE exp latency
hides under TensorE work.
"""

import sys

if "/opt/trn_rl_repo" not in sys.path:
    sys.path.insert(0, "/opt/trn_rl_repo")

import numpy as np

import concourse.bass as bass
import concourse.mybir as mybir
import concourse.tile as tile
from concourse import bacc
from concourse.bass import ts
from concourse.bass_utils import run_bass_kernel_spmd
from concourse.masks import make_identity

F32 = mybir.dt.float32
BF16 = mybir.dt.bfloat16
AF = mybir.ActivationFunctionType

L = 2048          # sequence length (both La and Lb)
D = 768           # model dim
H = 64            # projection dim
NT = L // 128     # 16 row/col tiles of 128
KD = D // 128     # 6 contraction tiles for the projections
NSUP = L // 512   # 4 supers (512-wide output stripes)
DP = D + 1        # moving operand width with the ones column

N_CORES = 8

_CACHE = {}


def _build():
    nc = bacc.Bacc("TRN2", target_bir_lowering=False, debug=False,
                   num_devices=N_CORES)
    A_d = nc.dram_tensor("A", [L, D], F32, kind="ExternalInput").ap()
    B_d = nc.dram_tensor("B", [L, D], F32, kind="ExternalInput").ap()
    WA_d = nc.dram_tensor("W_A", [D, H], F32, kind="ExternalInput").ap()
    WB_d = nc.dram_tensor("W_B", [D, H], F32, kind="ExternalInput").ap()
    bA_d = nc.dram_tensor("b_A", [H, 1], F32, kind="ExternalInput").ap()
    bB_d = nc.dram_tensor("b_B", [H, 1], F32, kind="ExternalInput").ap()
    AS_d = nc.dram_tensor("A_star", [L, D], F32, kind="ExternalOutput").ap()
    BS_d = nc.dram_tensor("B_star", [L, D], F32, kind="ExternalOutput").ap()

    with tile.TileContext(nc) as tc:
        with (
            tc.tile_pool(name="stage", bufs=2) as pstage,
            tc.tile_pool(name="mov", bufs=1) as pmov,
            tc.tile_pool(name="proj", bufs=1) as pproj,
            tc.tile_pool(name="pack", bufs=2) as ppack,
            tc.tile_pool(name="outp", bufs=4) as pout,
            tc.tile_pool(name="psum", bufs=2, space="PSUM") as pps,
        ):
            # identity for TensorE-based transposition
            ident = pmov.tile([128, 128], BF16, tag="ident", name="ident")
            make_identity(nc, ident)

            def load_weights():
                for side, (W_dram, b_dram) in (
                    ("B", (WB_d, bB_d)), ("A", (WA_d, bA_d))
                ):
                    wb = pmov.tile([128, KD, H], BF16, tag=f"w{side}",
                                   name=f"w{side}b")
                    nc.gpsimd.dma_start(
                        out=wb, in_=W_dram.rearrange("(k p) h -> p k h", p=128)
                    )
                    bt = pmov.tile([H, 1], F32, tag=f"b{side}",
                                   name=f"b{side}sb")
                    nc.gpsimd.dma_start(out=bt, in_=b_dram)
                    w_sb[side] = wb
                    b_sb[side] = bt

            w_sb = {}
            b_sb = {}

            # ---- load(+cast) and transpose via TensorE ----
            aug = {}
            projT = {}
            mts = {}
            dram = {"A": A_d, "B": B_d}
            for side in ("A", "B"):
                mts[side] = pmov.tile([128, NT * KD, 128], BF16,
                                      tag=f"t{side}", name=f"{side}_T")
                aug[side] = pmov.tile([128, NT, DP], BF16, tag=f"aug{side}",
                                      name=f"{side}_aug")

            def prep_unit(side, u):
                # 2-tile load unit (i = 2u, 2u+1): one casting DMA
                # f32 DRAM -> bf16 SBUF (SWDGE only); per-unit staging keeps
                # loads parallel (no false deps on big tensors)
                stg = pstage.tile([128, 2, D], BF16, tag="stg", bufs=6,
                                  name=f"stg{side}{u}")
                nc.gpsimd.dma_start(
                    out=stg,
                    in_=dram[side][u * 256:(u + 1) * 256, :].rearrange(
                        "(t p) d -> p t d", p=128
                    ),
                )
                for t in range(2):
                    i = 2 * u + t
                    # transpose the 6 blocks on TensorE: psum <- block.T
                    ps = pps.tile([128, 1024], F32, tag="accum",
                                  name=f"pstr{side}{i}")
                    for j in range(KD):
                        nc.tensor.matmul(ps[:, ts(j, 128)],
                                         stg[:, t, ts(j, 128)],
                                         ident, start=True, stop=True)
                    nc.vector.tensor_copy(
                        out=mts[side][:, i * KD:(i + 1) * KD, :],
                        in_=ps[:, 0:KD * 128],
                    )
                    nc.scalar.copy(out=aug[side][:, i, 0:D], in_=stg[:, t, :])

            def proj_chunk(side, n):
                # projT[h, s] = sum_d W[d,h] M^T[d,s] over 1024-wide chunk
                mtv = mts[side].rearrange("p (i j) q -> p i j q", j=KD)
                ps = pps.tile([128, 1024], F32, tag="spack",
                              name=f"psproj{side}{n}")
                for nn in range(2):     # 512-wide matmuls
                    i0 = n * 8 + nn * 4
                    for k in range(KD):
                        nc.tensor.matmul(
                            ps[:H, ts(nn, 512)],
                            w_sb[side][:, k, :],
                            mtv[:, i0:i0 + 4, k, :],
                            start=(k == 0), stop=(k == KD - 1),
                        )
                nc.scalar.activation(
                    out=projT[side][0:H, ts(n, 1024)], in_=ps[:H, :],
                    func=AF.Identity, bias=b_sb[side], scale=1.0,
                )
                # duplicate into partitions 64:128 for row-packed S matmuls
                nc.sync.dma_start(out=projT[side][64:128, ts(n, 1024)],
                                  in_=projT[side][0:H, ts(n, 1024)])

            for side in ("A", "B"):
                # rows 0:64 written by proj activation; rows 64:128 get a
                # duplicate (via SBUF->SBUF DMA) so K=64 score matmuls can be
                # row-packed two-at-a-time with tile_position (0,0)/(64,0)
                projT[side] = pproj.tile([128, L], BF16, tag=f"p{side}",
                                         name=f"{side}_projT")

            load_weights()
            for u in range(4):
                prep_unit("A", u)
            for u in range(8):
                prep_unit("B", u)
            nc.vector.memset(aug["B"][:, :, D:DP], 1.0)
            proj_chunk("B", 0)
            proj_chunk("B", 1)
            proj_chunk("A", 0)
            for u in range(4, 8):
                prep_unit("A", u)
            nc.vector.memset(aug["A"][:, :, D:DP], 1.0)
            proj_chunk("A", 1)

            # ---- main: per 512-wide output stripe, software-pipelined ----
            # dirn "A": produce A_star rows; panels are E'[t, s-stripe]
            #   (lhsT = B_projT tiles, rhs = A_projT stripe), moving = B_aug
            # dirn "B": produce B_star rows; panels are E[s, t-stripe]
            #   (lhsT = A_projT tiles, rhs = B_projT stripe), moving = A_aug
            work = [("A", u) for u in range(NSUP)] + \
                   [("B", u) for u in range(NSUP)]
            spec = {
                "A": (projT["B"], projT["A"], aug["B"], AS_d),
                "B": (projT["A"], projT["B"], aug["A"], BS_d),
            }
            packs = {}

            def emit_pack(w):
                dirn, u = w
                pT_l, pT_r, _, _ = spec[dirn]
                pk = ppack.tile([128, NT * 512], BF16, tag="pack",
                                name=f"pk{dirn}{u}")
                for jp in range(NT // 2):
                    ps = pps.tile([128, 1024], F32, tag="spack",
                                  name=f"pss{dirn}{u}{jp}")
                    for h2 in range(2):
                        # row-packed pair: K=64 matmuls in rows 0:64 / 64:128
                        j = jp * 2 + h2
                        base = h2 * 64
                        nc.tensor.matmul(
                            ps[:, ts(h2, 512)],
                            pT_l[base:base + H, ts(j, 128)],
                            pT_r[base:base + H, ts(u, 512)],
                            start=True, stop=True,
                            tile_position=(base, 0),
                        )
                    nc.scalar.activation(
                        out=pk[:, jp * 1024:(jp + 1) * 1024], in_=ps,
                        func=AF.Exp,
                    )
                packs[w] = pk

            def emit_accum(w):
                dirn, u = w
                _, _, mv, out_d = spec[dirn]
                pk = packs.pop(w)
                for ii in range(4):
                    pa = pps.tile([128, 1024], F32, tag="accum",
                                  name=f"pa{dirn}{u}{ii}")
                    for j in range(NT):
                        lhs = pk[:, j * 512 + ii * 128:
                                 j * 512 + ii * 128 + 128]
                        nc.tensor.matmul(
                            pa[:, 0:512], lhs, mv[:, j, 0:512],
                            start=(j == 0), stop=(j == NT - 1),
                        )
                        nc.tensor.matmul(
                            pa[:, 512:DP], lhs, mv[:, j, 512:DP],
                            start=(j == 0), stop=(j == NT - 1),
                        )
                    rinv = pout.tile([128, 1], F32, tag="rinv",
                                     name=f"ri{dirn}{u}{ii}")
                    nc.vector.reciprocal(out=rinv, in_=pa[:, D:DP])
                    ot = pout.tile([128, D], F32, tag="ot",
                                   name=f"ot{dirn}{u}{ii}")
                    nc.vector.tensor_scalar_mul(ot, pa[:, 0:D], rinv)
                    nc.sync.dma_start(
                        out=out_d[ts(u * 4 + ii, 128), :], in_=ot
                    )

            emit_pack(work[0])
            for idx, w in enumerate(work):
                if idx + 1 < len(work):
                    emit_pack(work[idx + 1])
                emit_accum(w)

    nc.compile()
    return nc


def _get_nc():
    if "nc" not in _CACHE:
        _CACHE["nc"] = _build()
    return _CACHE["nc"]


def _run(inputs, trace=False):
    nc = _get_nc()
    A = np.ascontiguousarray(np.asarray(inputs["A"], dtype=np.float32))
    B = np.ascontiguousarray(np.asarray(inputs["B"], dtype=np.float32))
    W_A = np.ascontiguousarray(np.asarray(inputs["W_A"], dtype=np.float32))
    W_B = np.ascontiguousarray(np.asarray(inputs["W_B"], dtype=np.float32))
    b_A = np.asarray(inputs["b_A"], dtype=np.float32).reshape(H, 1)
    b_B = np.asarray(inputs["b_B"], dtype=np.float32).reshape(H, 1)
    in_maps = [
        {
            "A": A[c], "B": B[c],
            "W_A": W_A, "W_B": W_B,
            "b_A": b_A, "b_B": b_B,
        }
        for c in range(N_CORES)
    ]
    res = run_bass_kernel_spmd(nc, in_maps, list(range(N_CORES)), trace=trace)
    A_star = np.stack([res.results[c]["A_star"] for c in range(N_CORES)])
    B_star = np.stack([res.results[c]["B_star"] for c in range(N_CORES)])
    return A_star, B_star, res


def kernel(**inputs):
    A_star, B_star, _ = _run(inputs)
    return A_star, B_star
